# revision 28
# baseline (speedup 1.0000x reference)
"""BioRNN Trainium2 kernel: 8-core tensor-parallel recurrence.

Strategy: column-shard the (coupling-folded, DECAY-prescaled, bf16) recurrent
weight matrix across 8 NeuronCores (512 output neurons each, N padded
3840->4096). All state is kept in transposed [neuron, batch] layout so every
elementwise op uses per-partition constants. Each step:
  AllGather bf16 rates -> 32 col-tiled matmuls (rT stationary [128,32],
  W moving [128,512], 4 interleaved PSUM strips) -> one fold-transpose matmul
  (strip-reduce + transpose in a single PE pass via a 0/1 fold matrix) ->
  epilogue (mGluR slow integration, leaky integration, rates) -> next step.
Readout (SR E-soma rates @ w_out) is computed redundantly on every core from
the gathered rates; core 0's copy is returned.
"""
import sys
sys.path.insert(0, '/opt/trn_rl_repo')
import numpy as np

import concourse.bass as bass
import concourse.mybir as mybir

try:
    import ml_dtypes
    BF16 = ml_dtypes.bfloat16
except ImportError:  # pragma: no cover
    import jax.numpy as jnp
    BF16 = jnp.bfloat16

# ---- model constants (hardcoded from the problem spec) ----
SIZES = [512, 1024, 128, 128, 128, 512, 1024, 128, 128, 128]
OFF = np.cumsum([0] + SIZES)
N = int(OFF[-1])            # 3840
NP_ = 4096                  # padded
N_BR = 2
N_IN, N_OUT = 128, 3
T_FULL, B = 100, 32
DECAY = np.float32(10.0 / 50.0)
NOISE_STD = 0.01
N_CORES = 8
SHARD = NP_ // N_CORES      # 512 neurons per core
NCH = NP_ // 128            # 32 k-chunks
CCH = SHARD // 128          # 4 chunks per core

_tau_me = np.tile(np.logspace(np.log10(100.0), np.log10(5000.0), SIZES[6] // N_BR), N_BR)
ALPHA_ME = (10.0 / _tau_me).astype(np.float32)

DT32 = mybir.dt.float32
DTBF = mybir.dt.bfloat16
AF = mybir.ActivationFunctionType
ALU = mybir.AluOpType


def build_kernel(T=T_FULL):
    import os
    variant = os.environ.get("BIO_VARIANT", "")
    nc = bass.Bass("TRN2", num_devices=N_CORES)

    # ---- DRAM parameters (per-core shards prepped on host) ----
    w_d = nc.declare_dram_parameter("w", [128, NCH * SHARD], DTBF, isOutput=False)
    win_d = nc.declare_dram_parameter("win", [128, SHARD], DTBF, isOutput=False)
    xt_d = nc.declare_dram_parameter("xt", [128, T * B], DTBF, isOutput=False)
    noise_d = nc.declare_dram_parameter("noise", [128, T * 128], DT32, isOutput=False)
    wout_d = nc.declare_dram_parameter("wout", [128, CCH * N_OUT], DTBF, isOutput=False)
    coef_d = nc.declare_dram_parameter("coef", [128, 3 * CCH], DT32, isOutput=False)
    dmask_d = nc.declare_dram_parameter("dmask", [128, SHARD // CCH], mybir.dt.uint8, isOutput=False)
    fmat_d = nc.declare_dram_parameter("fmat", [128, B], DTBF, isOutput=False)
    bout_d = nc.declare_dram_parameter("bout", [N_OUT, 1], DT32, isOutput=False)
    out_d = nc.declare_dram_parameter("out", [N_OUT, T * B], DT32, isOutput=True)

    # ---- collective bounce buffers ----
    in_b = [nc.dram_tensor(f"in_b{p}", [128, 128], DTBF) for p in range(2)]
    out_b = [nc.dram_tensor(f"out_b{p}", [128 * N_CORES, 128], DTBF, addr_space="Shared")
             for p in range(2)]

    FREE = SHARD // CCH  # 128 = CCH chunks x 32 batch in the free dim of state tiles

    from contextlib import ExitStack
    with ExitStack() as ctx:
        block = ctx.enter_context(nc.Block())
        sems = {n: ctx.enter_context(nc.semaphore(n)) for n in
                ["DINIT", "DO", "DO2", "DI", "DI2", "DI3", "DI4", "CC", "PEA", "PEF", "PEO",
                 "AC", "AI", "AR", "VH", "VR", "VO"]}
        DINIT = sems["DINIT"]; DO = sems["DO"]; DO2 = sems["DO2"]; DI = sems["DI"]; DI2 = sems["DI2"]; DI3 = sems["DI3"]; DI4 = sems["DI4"]; CC = sems["CC"]
        PEA = sems["PEA"]; PEF = sems["PEF"]; PEO = sems["PEO"]
        AC = sems["AC"]; AI = sems["AI"]; AR = sems["AR"]
        VH = sems["VH"]; VR = sems["VR"]; VO = sems["VO"]

        def sb(name, shape, dt):
            return ctx.enter_context(nc.sbuf_tensor(name, shape, dt))

        w_sb = sb("w_sb", [128, NCH * SHARD], DTBF)
        win_sb = sb("win_sb", [128, SHARD], DTBF)
        xt_sb = sb("xt_sb", [128, T * B], DTBF)
        noise_sb = sb("noise_sb", [128, T * 128], DT32)
        wout_sb = sb("wout_sb", [128, CCH * N_OUT], DTBF)
        coef_sb = sb("coef_sb", [128, 3 * CCH], DT32)
        dmask_sb = sb("dmask_sb", [128, FREE], mybir.dt.uint8)
        fmat_sb = sb("fmat_sb", [128, B], DTBF)
        bout_sb = sb("bout_sb", [N_OUT, 1], DT32)
        g_sb = sb("g_sb", [128, N_CORES * 128], DTBF)
        s_sb = sb("s_sb", [128, SHARD], DTBF)
        h_sb = sb("h_sb", [128, FREE], DT32)
        hn_sb = sb("hn_sb", [128, FREE], DT32)
        ime_sb = sb("ime_sb", [128, FREE], DT32)
        u_sb = sb("u_sb", [128, FREE], DT32)
        t2_sb = sb("t2_sb", [128, FREE], DT32)
        rs_sb = sb("rs_sb", [128, FREE], DT32)
        rr_sb = sb("rr_sb", [128, FREE], DT32)
        r_sb = sb("r_sb", [128, FREE], DTBF)
        o_sb = sb("o_sb", [N_OUT, T * B], DT32)
        ps1 = ctx.enter_context(nc.psum_tensor("ps1", [128, SHARD], DT32))
        ps2 = ctx.enter_context(nc.psum_tensor("ps2", [128, FREE], DT32))
        ps3 = ctx.enter_context(nc.psum_tensor("ps3", [N_OUT, B], DT32))

        N_INIT_DMA = 9

        @block.sync
        def _(sync):
            # init loads
            for dst, src in [
                (w_sb, w_d), (win_sb, win_d), (xt_sb, xt_d), (noise_sb, noise_d),
                (wout_sb, wout_d), (coef_sb, coef_d), (dmask_sb, dmask_d),
                (fmat_sb, fmat_d), (bout_sb, bout_d),
            ]:
                sync.dma_start(out=dst[:, :], in_=src[:, :]).then_inc(DINIT, 16)
            for t in range(T):
                p = t % 2
                # ship local rates shard (r_t) to bounce
                sync.dma_start(out=in_b[p][0:64, :], in_=r_sb[0:64, :]).wait_op(VR, t + 1, "sem-ge").then_inc(DO, 16)
                # pull gathered rates into SBUF, first half (ranks 0-3); the
                # second half goes in parallel on the scalar engine's queue
                if t > 0:
                    sync.wait_ge(PEO, t)
                for q, sem in [(0, DI), (1, DI2)]:
                    ob = out_b[p][256 * q:256 * (q + 1), :].rearrange("(c p) n -> p c n", p=128)
                    gb = g_sb[:, 256 * q:256 * (q + 1)].rearrange("p (c n) -> p c n", c=2)
                    d = sync.dma_start(out=gb, in_=ob).then_inc(sem, 16)
                    if q == 0:
                        d.wait_op(CC, t + 1, "sem-ge")
            # final output store
            sync.wait_ge(VO, T)
            sync.dma_start(out=out_d[:, :], in_=o_sb[:, :]).then_inc(DO2, 16)

        @block.gpsimd
        def _(gpsimd):
            for t in range(T):
                p = t % 2
                if variant == "noag":
                    gpsimd.wait_ge(DO, 32 * (t + 1))
                    gpsimd.sem_inc(CC, 1)
                else:
                    gpsimd.collective_compute(
                        "AllGather",
                        ALU.bypass,
                        replica_groups=[list(range(N_CORES))],
                        ins=[in_b[p].ap().opt()],
                        outs=[out_b[p].ap().opt()],
                    ).wait_op(DO, 32 * (t + 1), "sem-ge").then_inc(CC)

        @block.tensor
        def _(pe):
            pe.wait_ge(DINIT, 16 * N_INIT_DMA)
            for t in range(T):
                # x_t contribution into strip 0 (runs during the AllGather)
                if t > 0:
                    pe.wait_ge(AC, 4 * t)  # ps1 free: ACT copy of prev step done
                nc.tensor.matmul(
                    out=ps1[0:32, :],
                    lhsT=xt_sb[:, B * t:B * (t + 1)],
                    rhs=win_sb[:, :],
                    start=True, stop=False,
                    tile_position=(0, 0),
                    skip_group_check=True,
                )
                # main recurrent matmuls: 8 groups x 4 col-tiled strips
                pe.wait_ge(DI, 16 * (t + 1))
                waited = {0}
                for g in range(0 if variant == "nomm" else NCH // 4):
                    q = g // 2
                    if q not in waited:
                        pe.wait_ge([DI, DI2, DI3, DI4][q], 16 * (t + 1))
                        waited.add(q)
                    for j in range(4):
                        kc = 4 * g + j
                        mm = nc.tensor.matmul(
                            out=ps1[32 * j:32 * (j + 1), :],
                            lhsT=g_sb[:, 32 * kc:32 * (kc + 1)],
                            rhs=w_sb[:, SHARD * kc:SHARD * (kc + 1)],
                            start=(g == 0 and j != 0),
                            stop=(g == NCH // 4 - 1),
                            skip_group_check=True,
                            tile_position=(0, 32 * j),
                        )
                if variant == "nomm":
                    nc.tensor.matmul(out=ps1[0:32, 0:32], lhsT=xt_sb[:, 0:32],
                                     rhs=win_sb[:, 0:32], start=False, stop=False,
                                     skip_group_check=True).then_inc(PEA, 1)
                else:
                    mm.then_inc(PEA, 1)
                # fold-transpose: strip-reduce + transpose via 0/1 fold matrix
                for c in range(CCH):
                    pe.wait_ge(AC, 4 * t + c + 1)
                    mm = nc.tensor.matmul(
                        out=ps2[:, B * c:B * (c + 1)],
                        lhsT=s_sb[:, 128 * c:128 * (c + 1)],
                        rhs=fmat_sb[:, :],
                        start=(c == 0), stop=(c == CCH - 1),
                    )
                mm.then_inc(PEF, 1)
                # readout: out_t = r_t[SR_ES] @ w_out  (chunks 0-3 of the gather)
                if t > 0:
                    pe.wait_ge(VO, t)  # ps3 free
                for c in range(CCH):
                    mm = nc.tensor.matmul(
                        out=ps3[:, :],
                        lhsT=wout_sb[:, N_OUT * c:N_OUT * (c + 1)],
                        rhs=g_sb[:, 32 * c:32 * (c + 1)],
                        start=(c == 0), stop=(c == CCH - 1),
                    )
                mm.then_inc(PEO, 1)

        @block.scalar
        def _(act):
            act.wait_ge(DINIT, 16 * N_INIT_DMA)
            # r_0 from h_0 = 0
            act.wait_ge(VH, 1)
            nc.scalar.activation(rs_sb[:, :], h_sb[:, :], AF.Sigmoid)
            nc.scalar.activation(rr_sb[:, :], h_sb[:, :], AF.Relu).then_inc(AR, 1)
            for t in range(T):
                # shard-store half 2 (partitions 64-127) in parallel with sync's
                p = t % 2
                act.dma_start(out=in_b[p][64:128, :], in_=r_sb[64:128, :]).wait_op(VR, t + 1, "sem-ge").then_inc(DO, 16)
                # second-half gather load (ranks 4-7) in parallel with sync's half
                if t > 0:
                    act.wait_ge(PEO, t)
                for q, sem in [(2, DI3), (3, DI4)]:
                    ob2 = out_b[p][256 * q:256 * (q + 1), :].rearrange("(c p) n -> p c n", p=128)
                    gb2 = g_sb[:, 256 * q:256 * (q + 1)].rearrange("p (c n) -> p c n", c=2)
                    d2 = act.dma_start(out=gb2, in_=ob2).then_inc(sem, 16)
                    if q == 2:
                        d2.wait_op(CC, t + 1, "sem-ge")
                # psum1 strips -> SBUF bf16, 4 column chunks (pipelines with fold MMs)
                for c in range(CCH):
                    cp = nc.scalar.copy(
                        out=s_sb[:, 128 * c:128 * (c + 1)],
                        in_=ps1[:, 128 * c:128 * (c + 1)],
                    ).then_inc(AC, 1)
                    if c == 0:
                        cp.wait_op(PEA, t + 1, "sem-ge")
                # mGluR: t2 = relu(alpha * pre + beta) per chunk
                for c in range(CCH):
                    a2 = nc.scalar.activation(
                        t2_sb[:, B * c:B * (c + 1)],
                        ps2[:, B * c:B * (c + 1)],
                        AF.Relu,
                        scale=coef_sb[:, c:c + 1],
                        bias=coef_sb[:, 2 * CCH + c:2 * CCH + c + 1],
                    ).then_inc(AI, 1)
                    if c == 0:
                        a2.wait_op(PEF, t + 1, "sem-ge")
                # rates nonlinearities for h_{t+1}
                nc.scalar.activation(rs_sb[:, :], h_sb[:, :], AF.Sigmoid).wait_op(VH, t + 2, "sem-ge")
                nc.scalar.activation(rr_sb[:, :], h_sb[:, :], AF.Relu).then_inc(AR, 1)

        @block.vector
        def _(dve):
            dve.wait_ge(DINIT, 16 * N_INIT_DMA)
            dve.memset(h_sb[:, :], 0.0)
            dve.memset(ime_sb[:, :], 0.0).then_inc(VH, 1)
            # r_0
            dve.wait_ge(AR, 1)
            nc.vector.select(r_sb[:, :], dmask_sb[:, :], rs_sb[:, :], rr_sb[:, :], add_drain=True).then_inc(VR, 1)
            for t in range(T):
                # hn = 0.8*h + noise'_t  (runs during the AllGather)
                nc.vector.scalar_tensor_tensor(
                    out=hn_sb[:, :], in0=h_sb[:, :], scalar=float(1.0 - DECAY),
                    in1=noise_sb[:, 128 * t:128 * (t + 1)], op0=ALU.mult, op1=ALU.add,
                )
                # ime = (1-alpha)*ime + t2   per chunk
                def ime_upd(c):
                    nc.vector.scalar_tensor_tensor(
                        out=ime_sb[:, B * c:B * (c + 1)],
                        in0=ime_sb[:, B * c:B * (c + 1)],
                        scalar=coef_sb[:, CCH + c:CCH + c + 1],
                        in1=t2_sb[:, B * c:B * (c + 1)],
                        op0=ALU.mult, op1=ALU.add,
                    ).wait_op(AI, 4 * t + c + 1, "sem-ge")
                for c in range(CCH):
                    ime_upd(c)
                dve.drain()
                nc.vector.tensor_tensor(
                    out=u_sb[:, :], in0=hn_sb[:, :], in1=ps2[:, :], op=ALU.add)
                dve.drain()
                nc.vector.tensor_tensor(
                    out=h_sb[:, :], in0=u_sb[:, :], in1=ime_sb[:, :], op=ALU.add,
                ).then_inc(VH, 1)
                # r_{t+1}
                dve.wait_ge(AR, t + 2)
                nc.vector.select(
                    r_sb[:, :], dmask_sb[:, :], rs_sb[:, :], rr_sb[:, :], add_drain=True
                ).then_inc(VR, 1)
                # readout add bias
                nc.vector.tensor_scalar(
                    out=o_sb[:, B * t:B * (t + 1)], in0=ps3[:, :],
                    scalar1=bout_sb[:, 0:1], scalar2=None, op0=ALU.add,
                ).wait_op(PEO, t + 1, "sem-ge").then_inc(VO, 1)

    return nc


# ---------------- host-side prep ----------------

def _to_bf16(a):
    return np.asarray(a, np.float32).astype(BF16)


def prep_inputs(x, noise, w_rec, w_in, b, d2s, w_out, b_out, mask, T=T_FULL):
    x = np.asarray(x, np.float32)[:T]
    noise = np.asarray(noise, np.float32)[:T]
    w_rec = np.asarray(w_rec, np.float32)
    w_in = np.asarray(w_in, np.float32)
    b = np.asarray(b, np.float32)
    d2s = np.asarray(d2s, np.float32)
    w_out = np.asarray(w_out, np.float32)
    b_out = np.asarray(b_out, np.float32)
    mask = np.asarray(mask, np.float32)

    # effective recurrent weights with dend->soma coupling folded in, DECAY-scaled
    W = np.zeros((NP_, NP_), np.float32)
    W[:N, :N] = np.abs(w_rec) * mask
    d2s_sr = d2s[:SIZES[1]].reshape(N_BR, SIZES[0])
    d2s_pfc = d2s[SIZES[1]:].reshape(N_BR, SIZES[5])
    for k in range(N_BR):
        W[np.arange(OFF[1] + k * SIZES[0], OFF[1] + (k + 1) * SIZES[0]),
          np.arange(OFF[0], OFF[1])] += d2s_sr[k]
        W[np.arange(OFF[6] + k * SIZES[5], OFF[6] + (k + 1) * SIZES[5]),
          np.arange(OFF[5], OFF[6])] += d2s_pfc[k]
    W *= DECAY
    Wb = _to_bf16(W)                       # [4096, 4096]

    win_full = np.zeros((N_IN, NP_), np.float32)
    win_full[:, :N] = w_in * DECAY
    winb = _to_bf16(win_full)

    # per-(neuron) coefficient vectors, padded
    alpha = np.zeros(NP_, np.float32)
    alpha[OFF[6]:OFF[7]] = ALPHA_ME
    beta = alpha * DECAY * np.pad(b, (0, NP_ - N))   # bias term inside the relu
    dend = np.zeros(NP_, np.float32)
    dend[OFF[1]:OFF[2]] = 1.0
    dend[OFF[6]:OFF[7]] = 1.0

    ns = np.float32(np.float32(np.sqrt(2.0 * DECAY)) * np.float32(NOISE_STD))
    # noise' = ns*noise + DECAY*b  (pre-scaled, transposed, padded)
    noise_p = np.zeros((T, B, NP_), np.float32)
    noise_p[:, :, :N] = ns * noise
    noise_p += (DECAY * np.pad(b, (0, NP_ - N)))[None, None, :]

    # xt layout [128 part = N_IN, T*B]: xt[p, 32t+b] = x[t, b, p]
    xt = np.transpose(x, (2, 0, 1)).reshape(N_IN, T * B)
    xtb = _to_bf16(xt)

    # fold matrix [128, 32]: F[32j+b, b] = 1
    F = np.zeros((128, B), np.float32)
    for j in range(4):
        F[32 * j + np.arange(B), np.arange(B)] = 1.0
    Fb = _to_bf16(F)

    wout_p = np.zeros((SIZES[0], N_OUT), np.float32)
    wout_p[:] = w_out
    woutb = _to_bf16(wout_p.reshape(CCH, 128, N_OUT))   # [4, 128, 3]

    in_maps = []
    for core in range(N_CORES):
        cols = slice(SHARD * core, SHARD * (core + 1))
        # w: [128, kc*SHARD]: w[p, SHARD*kc+n] = W[128kc+p, 512core+n]
        wshard = np.ascontiguousarray(
            Wb[:, cols].reshape(NCH, 128, SHARD).transpose(1, 0, 2).reshape(128, NCH * SHARD))
        winshard = np.ascontiguousarray(winb[:, cols])
        # noise: [128, T*128]: noise[p, 128t+32c+b] = noise_p[t, b, 512core+128c+p]
        nshard = noise_p[:, :, cols].reshape(T, B, CCH, 128)
        nshard = np.ascontiguousarray(nshard.transpose(3, 0, 2, 1).reshape(128, T * CCH * B))
        # coef [128, 12]: alpha(4) | 1-alpha(4) | beta(4)
        a_sh = alpha[cols].reshape(CCH, 128).T        # [128, 4]
        b_sh = beta[cols].reshape(CCH, 128).T
        coef = np.concatenate([a_sh, 1.0 - a_sh, b_sh], axis=1).astype(np.float32)
        dm = np.repeat(dend[cols].reshape(CCH, 128).T[:, :, None], B, axis=2).reshape(128, CCH * B)
        in_maps.append({
            "w": wshard,
            "win": winshard,
            "xt": xtb,
            "noise": nshard,
            "wout": np.ascontiguousarray(woutb.transpose(1, 0, 2).reshape(128, CCH * N_OUT)),
            "coef": coef,
            "dmask": np.ascontiguousarray(dm.astype(np.uint8)),
            "fmat": Fb,
            "bout": b_out.reshape(N_OUT, 1).astype(np.float32),
        })
    return in_maps


def unshard(out_core0, T=T_FULL):
    # out [3, T*B] -> [T, B, 3]
    o = np.asarray(out_core0, np.float32).reshape(N_OUT, T, B)
    return np.ascontiguousarray(o.transpose(1, 2, 0))


# ---------------- runner (inline; kernel.py must be self-contained) ----------------

_CACHE = {}


def _get_runner(T=T_FULL):
    if T in _CACHE:
        return _CACHE[T]
    import jax
    from jax.sharding import Mesh, PartitionSpec, NamedSharding
    from jax.experimental.shard_map import shard_map
    from concourse.bass2jax import _bass_exec_p, install_neuronx_cc_hook, partition_id_tensor

    install_neuronx_cc_hook()
    nc = build_kernel(T)

    partition_name = nc.partition_id_tensor.name if nc.partition_id_tensor else None
    in_names, out_names, out_avals, zero_outs = [], [], [], []
    for alloc in nc.m.functions[0].allocations:
        if not isinstance(alloc, mybir.MemoryLocationSet):
            continue
        name = alloc.memorylocations[0].name
        if alloc.kind == "ExternalInput":
            if name != partition_name and (nc.dbg_addr is None or name != nc.dbg_addr.name):
                in_names.append(name)
        elif alloc.kind == "ExternalOutput":
            out_names.append(name)
            shape = tuple(alloc.tensor_shape)
            dtype = mybir.dt.np(alloc.dtype)
            out_avals.append(jax.core.ShapedArray(shape, dtype))
            zero_outs.append(np.zeros(shape, dtype))
    n_params = len(in_names)
    all_in_names = list(in_names) + list(out_names)
    has_dbg = nc.dbg_addr is not None
    if has_dbg:
        all_in_names.append(nc.dbg_addr.name)
    if partition_name is not None:
        all_in_names.append(partition_name)

    def _body(*args):
        operands = list(args)
        if has_dbg:
            operands.append(jax.numpy.zeros((1, 2), jax.numpy.uint32))
        if partition_name is not None:
            operands.append(partition_id_tensor())
        return tuple(_bass_exec_p.bind(
            *operands,
            out_avals=tuple(out_avals),
            in_names=tuple(all_in_names),
            out_names=tuple(out_names),
            lowering_input_output_aliases=(),
            sim_require_finite=True,
            sim_require_nnan=True,
            nc=nc,
        ))

    devices = jax.devices()[:N_CORES]
    mesh = Mesh(np.asarray(devices), ("core",))
    n_outs = len(out_names)
    sharded = jax.jit(
        shard_map(_body, mesh=mesh,
                  in_specs=(PartitionSpec("core"),) * (n_params + n_outs),
                  out_specs=(PartitionSpec("core"),) * n_outs,
                  check_rep=False),
        keep_unused=True,
    )
    sharding = NamedSharding(mesh, PartitionSpec("core"))
    state = dict(nc=nc, in_names=in_names, out_names=out_names, out_avals=out_avals,
                 zero_outs=zero_outs, sharded=sharded, sharding=sharding, mesh=mesh)
    _CACHE[T] = state
    return state


def run_device(in_maps, T=T_FULL, stage=None):
    import jax
    st = _get_runner(T)
    sharding = st["sharding"]
    concat_in = [
        jax.device_put(np.concatenate([np.asarray(m[name]) for m in in_maps], axis=0), sharding)
        for name in st["in_names"]
    ]
    concat_zeros = [
        jax.device_put(np.zeros((N_CORES * z.shape[0], *z.shape[1:]), z.dtype), sharding)
        for z in st["zero_outs"]
    ]
    out_arrs = st["sharded"](*concat_in, *concat_zeros)
    jax.block_until_ready(out_arrs)
    # core 0's "out"
    i = st["out_names"].index("out")
    full = np.asarray(out_arrs[i])
    per_core_rows = st["out_avals"][i].shape[0]
    return full[:per_core_rows]


def kernel(**inputs):
    in_maps = prep_inputs(**inputs)
    out0 = run_device(in_maps, T=T_FULL)
    return unshard(out0, T=T_FULL)


if __name__ == "__main__":
    nc = build_kernel(4)
    print("build OK")
